# revision 1
# baseline (speedup 1.0000x reference)
"""Trainium2 Bass kernel for CustomMamba (data-parallel over (b*n) scans).

Self-contained: builds + compiles a single-core SPMD Bass/Tile program,
shards inputs over 8 NeuronCores (along n), runs via run_bass_kernel_spmd,
and gathers the full output.
"""

import sys

sys.path.insert(0, "/opt/trn_rl_repo")

import os

os.environ.setdefault("JAX_PLATFORMS", "cpu")

from contextlib import ExitStack

import numpy as np

import concourse.bacc as bacc
import concourse.mybir as mybir
from concourse.bass_utils import run_bass_kernel_spmd
from concourse.masks import make_identity
from concourse.tile import TileContext
from concourse._compat import axon_active

FP = mybir.dt.float32
AF = mybir.ActivationFunctionType
OP = mybir.AluOpType

# Problem constants (hardcoded per spec)
B, T, N, F = 8, 24, 512, 64
DI, DS, DR, DC = 128, 16, 4, 4
NCORES = 8


def _host_consts(inputs):
    """Fold the linear layers into per-stage weight matrices (fp32 numpy)."""
    w_mix = np.asarray(inputs["w_mix"], np.float32)      # [2F, F]
    b_mix = np.asarray(inputs["b_mix"], np.float32)      # [F]
    w_in = np.asarray(inputs["w_in"], np.float32)        # [F, 2*DI]
    conv_w = np.asarray(inputs["conv_w"], np.float32)    # [DI, DC]
    conv_b = np.asarray(inputs["conv_b"], np.float32)    # [DI]
    w_xproj = np.asarray(inputs["w_xproj"], np.float32)  # [DI, DR+2*DS]
    w_dt = np.asarray(inputs["w_dt"], np.float32)        # [DR, DI]
    b_dt = np.asarray(inputs["b_dt"], np.float32)        # [DI]
    A_log = np.asarray(inputs["A_log"], np.float32)      # [DI, DS]
    D = np.asarray(inputs["D"], np.float32)              # [DI]
    w_out = np.asarray(inputs["w_out"], np.float32)      # [DI, F]

    W1 = w_mix @ w_in                                    # [128, 2*DI]
    b1 = b_mix @ w_in                                    # [2*DI]
    W1x, W1z = W1[:, :DI].copy(), W1[:, DI:].copy()
    b1x, b1z = b1[:DI].copy(), b1[DI:].copy()

    W2dt = (w_xproj[:, :DR] @ w_dt).copy()               # [DI, DI]
    W2bc = w_xproj[:, DR:].copy()                        # [DI, 2*DS]

    A = -np.exp(A_log)                                   # [DI, DS]
    assert np.allclose(A, A[0:1, :], rtol=1e-6), "A varies across d"
    A_s = [float(A[0, s]) for s in range(DS)]

    return dict(
        W1x=W1x, W1z=W1z, b1x=b1x, b1z=b1z,
        W2dt=W2dt, W2bc=W2bc, b_dt=b_dt,
        conv_w=conv_w, conv_b=conv_b, D=D, w_out=w_out, A_s=A_s,
    )


def build_program(n_c, consts, cmul_engine="vector", debug=None,
                  x_eng="gpsimd", add_eng="vector", sbufs=2):
    """Build + compile the per-core Bass program. n_c = n-shard width."""
    if debug is None:
        debug = not axon_active()
    nc = bacc.Bacc(
        "TRN2",
        target_bir_lowering=False,
        debug=debug,
        enable_asserts=True,
        num_devices=1,
    )

    bn = B * n_c
    ic = min(128, bn)
    nblk = bn // ic
    assert nblk * ic == bn
    bpb = ic // n_c                    # b's per block
    assert bpb * n_c == ic and bpb >= 1
    CT = ic * T

    x_d = nc.dram_tensor("x_sh", (B, T, n_c, F), FP, kind="ExternalInput").ap()
    qk_d = nc.dram_tensor("qk_sh", (B, T, n_c, F), FP, kind="ExternalInput").ap()
    cd = {}
    for nm, shp in [
        ("W1x", (2 * F, DI)), ("W1z", (2 * F, DI)),
        ("b1x", (DI, 1)), ("b1z", (DI, 1)),
        ("W2dt", (DI, DI)), ("W2bc", (DI, 2 * DS)), ("b_dt", (DI, 1)),
        ("conv_w", (DI, DC)), ("conv_b", (DI, 1)), ("D", (DI, 1)),
        ("w_out", (DI, F)),
    ]:
        cd[nm] = nc.dram_tensor(nm, shp, FP, kind="ExternalInput").ap()
    out_d = nc.dram_tensor("out_sh", (B, T, n_c, F), FP, kind="ExternalOutput").ap()
    z_sp = nc.dram_tensor("z_spill", (nblk, DI, CT), FP, kind="Internal").ap()
    xc_sp = nc.dram_tensor("xc_spill", (nblk, DI, CT), FP, kind="Internal").ap()

    with TileContext(nc) as tc:
        _body(nc, tc, x_d, qk_d, cd, out_d, z_sp, xc_sp,
              n_c, ic, nblk, bpb, CT, consts, cmul_engine, x_eng, add_eng,
              sbufs)
    nc.compile()
    return nc


def _body(nc, tc, x_d, qk_d, cd, out_d, z_sp, xc_sp,
          n_c, ic, nblk, bpb, CT, consts, cmul_engine, x_eng="vector",
          add_eng="vector", sbufs=2):
    P = ic
    DH = 64                            # d-half width for scan-phase tiles
    NDH = DI // DH
    NMM = 512 if CT % 512 == 0 else CT  # matmul N-chunk
    TG = 4                             # t's merged per transpose-psum tile
    use_b1 = not (np.allclose(consts["b1x"], 0) and np.allclose(consts["b1z"], 0))
    use_cb = not np.allclose(consts["conv_b"], 0)
    A_s = consts["A_s"]

    es = ExitStack()
    sb = es.enter_context(tc.tile_pool(name="sb", bufs=1))
    sb2 = es.enter_context(tc.tile_pool(name="sb2", bufs=sbufs))
    ps = es.enter_context(tc.tile_pool(name="ps", bufs=2, space="PSUM"))

    # ---- constants ----
    ct = {}
    for nm in cd:
        t = sb.tile(list(cd[nm].shape), FP, tag=f"c_{nm}")
        nc.sync.dma_start(t[:], cd[nm])
        ct[nm] = t
    ident = sb.tile([128, 128], FP, tag="ident")
    make_identity(nc, ident[:])

    BSUB = min(bpb, 128 // T)          # b's per load/transpose chunk

    for blk in range(nblk):
        b0 = blk * bpb

        # ---- load + transpose x/qk into xcatT [128=(fx|fqk), (i,t)] ----
        xcatT = sb.tile([128, CT], FP, tag="xcatT")
        NG = 8                         # n's per transpose-psum group
        for c0 in range(0, bpb, BSUB):
            bs = min(BSUB, bpb - c0)
            nrow = bs * T
            slot = 64 if nrow <= 64 else 128   # bank-aligned transpose slots
            for src_d, half, tagr in ((x_d, 0, "xraw"), (qk_d, 1, "qraw")):
                raw = sb.tile([nrow, n_c * F], FP, tag=tagr)
                nc.sync.dma_start(
                    raw[:],
                    src_d[b0 + c0:b0 + c0 + bs].rearrange(
                        "b t n f -> (b t) (n f)"),
                )
                for g in range((n_c + NG - 1) // NG):
                    ng = min(NG, n_c - g * NG)
                    pt = ps.tile([F, NG * slot], FP, tag="tps")
                    for k in range(ng):
                        n_ = g * NG + k
                        nc.tensor.transpose(
                            pt[:, k * slot:k * slot + nrow],
                            raw[:, n_ * F:(n_ + 1) * F],
                            ident[:nrow, :nrow],
                        )
                    # psum [F, (n ng, b bs, t T)] -> xcatT cols (b*n_c+n)*T+t
                    dst = xcatT[half * F:(half + 1) * F, :].rearrange(
                        "p (i t) -> p i t", t=T
                    ).rearrange("p (b n) t -> p n b t", b=bpb)[
                        :, g * NG:g * NG + ng, c0:c0 + bs]
                    src_ap = pt[:].rearrange(
                        "p (n r) -> p n r", r=slot)[:, :ng, :nrow].rearrange(
                        "p n (b t) -> p n b t", t=T)
                    nc.scalar.copy(out=dst, in_=src_ap)

        # ---- M1: xc = W1x.T @ xcatT ; z = W1z.T @ xcatT ----
        xc = sb.tile([DI, CT], FP, tag="xc")
        z = sb.tile([DI, CT], FP, tag="z")
        for c0 in range(0, CT, NMM):
            pxc = ps.tile([DI, NMM], FP, tag="m1a")
            pz = ps.tile([DI, NMM], FP, tag="m1b")
            nc.tensor.matmul(pxc[:], ct["W1x"][:], xcatT[:, c0:c0 + NMM],
                             start=True, stop=True)
            nc.tensor.matmul(pz[:], ct["W1z"][:], xcatT[:, c0:c0 + NMM],
                             start=True, stop=True)
            if use_b1:
                nc.scalar.activation(xc[:, c0:c0 + NMM], pxc[:], AF.Identity,
                                     bias=ct["b1x"][:, 0:1])
                nc.scalar.activation(z[:, c0:c0 + NMM], pz[:], AF.Identity,
                                     bias=ct["b1z"][:, 0:1])
            else:
                nc.scalar.copy(out=xc[:, c0:c0 + NMM], in_=pxc[:])
                nc.scalar.copy(out=z[:, c0:c0 + NMM], in_=pz[:])
        nc.sync.dma_start(z_sp[blk], z[:])

        # ---- causal depthwise conv (+bias) + silu ----
        acc = sb.tile([DI, CT], FP, tag="acc")
        nc.scalar.mul(acc[:], xc[:], ct["conv_w"][:, DC - 1:DC])
        xc3 = xc[:].rearrange("p (i t) -> p i t", t=T)
        ac3 = acc[:].rearrange("p (i t) -> p i t", t=T)
        for k in range(DC - 1):
            d = DC - 1 - k
            nc.vector.scalar_tensor_tensor(
                out=ac3[:, :, d:], in0=xc3[:, :, :T - d],
                scalar=ct["conv_w"][:, k:k + 1],
                in1=ac3[:, :, d:], op0=OP.mult, op1=OP.add,
            )
        xc2 = acc
        if use_cb:
            nc.scalar.activation(acc[:], acc[:], AF.Identity,
                                 bias=ct["conv_b"][:, 0:1])
        # silu(v) = v * sigmoid(v); Silu itself is absent from CoreSim
        sg = sb.tile([DI, CT], FP, tag="xcatT")
        nc.scalar.activation(sg[:], acc[:], AF.Sigmoid)
        nc.vector.tensor_tensor(xc2[:], acc[:], sg[:], OP.mult)

        # ---- M2: dt = softplus(W2dt.T @ xc2 + b_dt); bc = W2bc.T @ xc2 ----
        dt = sb.tile([DI, CT], FP, tag="z")      # z already spilled
        bc = sb.tile([2 * DS, CT], FP, tag="m2tmp")
        for c0 in range(0, CT, NMM):
            pdt = ps.tile([DI, NMM], FP, tag="m1a")
            pbc = ps.tile([2 * DS, NMM], FP, tag="m1b")
            nc.tensor.matmul(pdt[:], ct["W2dt"][:], xc2[:, c0:c0 + NMM],
                             start=True, stop=True)
            nc.tensor.matmul(pbc[:], ct["W2bc"][:], xc2[:, c0:c0 + NMM],
                             start=True, stop=True)
            # softplus(x + b_dt) = ln(1 + exp(x + b_dt)); Softplus has no
            # activation table on gen3, but Exp and Ln share one.
            spe = sb2.tile([DI, NMM], FP, tag="spe")
            nc.scalar.activation(spe[:], pdt[:], AF.Exp,
                                 bias=ct["b_dt"][:, 0:1])
            nc.scalar.activation(dt[:, c0:c0 + NMM], spe[:], AF.Ln, bias=1.0)
            nc.scalar.copy(out=bc[:, c0:c0 + NMM], in_=pbc[:])

        du = sb.tile([DI, CT], FP, tag="du")
        nc.vector.tensor_tensor(du[:], dt[:], xc2[:], OP.mult)
        nc.sync.dma_start(xc_sp[blk], xc2[:])

        # ---- transpose dt,du -> [i,(d,t)]; bc -> [i,(sc,t)] ----
        dtT = sb.tile([P, DI * T], FP, tag="dtT")
        duT = sb.tile([P, DI * T], FP, tag="duT")
        bcT = sb.tile([P, 2 * DS * T], FP, tag="bcT")
        for (srct, dstt, rows) in ((dt, dtT, DI), (du, duT, DI),
                                   (bc, bcT, 2 * DS)):
            s3 = srct[:].rearrange("p (i t) -> p i t", t=T)
            for t0 in range(0, T, TG):
                pt = ps.tile([P, TG * rows], FP, tag="tps")
                for k in range(TG):
                    nc.tensor.transpose(
                        pt[:, k * rows:(k + 1) * rows],
                        s3[:rows, :, t0 + k],
                        ident[:rows, :rows],
                    )
                dst = dstt[:].rearrange("p (d t) -> p d t", t=T)[:, :, t0:t0 + TG]
                nc.scalar.copy(
                    out=dst, in_=pt[:].rearrange("p (t d) -> p d t", t=TG))

        # ---- scan phase ----
        y_d = sb.tile([DI, CT], FP, tag="du")    # reuse du slot post-transpose
        duT3 = duT[:].rearrange("p (d t) -> p d t", t=T)
        bcT3 = bcT[:].rearrange("p (c t) -> p c t", t=T)
        cmul = nc.vector if cmul_engine == "vector" else nc.gpsimd
        for dh in range(NDH):
            d0 = dh * DH
            ya = None
            for s in range(DS):
                dA = sb2.tile([P, DH * T], FP, tag="dA")
                Xs = sb2.tile([P, DH * T], FP, tag="Xs")
                nc.scalar.activation(dA[:], dtT[:, d0 * T:(d0 + DH) * T],
                                     AF.Exp, scale=A_s[s])
                dA3 = dA[:].rearrange("p (d t) -> p d t", t=T)
                nc.gpsimd.memset(dA3[:, :, 0:1], 0.0)
                xeng = nc.vector if x_eng == "vector" else nc.gpsimd
                xeng.tensor_tensor(
                    Xs[:].rearrange("p (d t) -> p d t", t=T),
                    duT3[:, d0:d0 + DH],
                    bcT3[:, s:s + 1, :].to_broadcast((P, DH, T)),
                    OP.mult,
                )
                hs = sb2.tile([P, DH * T], FP, tag="dA")
                nc.vector.tensor_tensor_scan(hs[:], dA[:], Xs[:], 0.0,
                                             OP.mult, OP.add)
                tmp = sb2.tile([P, DH * T], FP, tag="Xs")
                cmul.tensor_tensor(
                    tmp[:].rearrange("p (d t) -> p d t", t=T),
                    hs[:].rearrange("p (d t) -> p d t", t=T),
                    bcT3[:, DS + s:DS + s + 1, :].to_broadcast((P, DH, T)),
                    OP.mult,
                )
                yb = sb2.tile([P, DH * T], FP, tag="yp")
                if ya is None:
                    nc.vector.tensor_copy(out=yb[:], in_=tmp[:])
                else:
                    if add_eng == "vector":
                        eng = nc.vector
                    elif add_eng == "gpsimd":
                        eng = nc.gpsimd
                    else:
                        eng = nc.vector if (s % 2 == 0) else nc.gpsimd
                    eng.tensor_tensor(yb[:], ya[:], tmp[:], OP.add)
                ya = yb
            # transpose y [i,(d-half,t)] back into y_d [d,(i,t)]
            ya3 = ya[:].rearrange("p (d t) -> p d t", t=T)
            for t0 in range(0, T, TG):
                pt = ps.tile([DH, TG * P], FP, tag="tps")
                for k in range(TG):
                    nc.tensor.transpose(pt[:, k * P:(k + 1) * P],
                                        ya3[:, :, t0 + k], ident[:P, :P])
                dst = y_d[d0:d0 + DH, :].rearrange(
                    "p (i t) -> p i t", t=T)[:, :, t0:t0 + TG]
                nc.scalar.copy(out=dst,
                               in_=pt[:].rearrange("p (t i) -> p i t", t=TG))

        # ---- gate: y2 = (y_d + xc2*D) * silu(z) ----
        zr = sb.tile([DI, CT], FP, tag="z")
        xcr = sb.tile([DI, CT], FP, tag="acc")
        nc.sync.dma_start(zr[:], z_sp[blk])
        nc.sync.dma_start(xcr[:], xc_sp[blk])
        sz = sb.tile([DI, CT], FP, tag="sz")
        sg2 = sb.tile([DI, CT], FP, tag="xcatT")
        nc.scalar.activation(sg2[:], zr[:], AF.Sigmoid)
        nc.vector.tensor_tensor(sz[:], zr[:], sg2[:], OP.mult)
        nc.vector.scalar_tensor_tensor(
            out=y_d[:], in0=xcr[:], scalar=ct["D"][:, 0:1],
            in1=y_d[:], op0=OP.mult, op1=OP.add,
        )
        nc.vector.tensor_tensor(sz[:], y_d[:], sz[:], OP.mult)

        # ---- out = w_out.T @ y2 ; per-(b,t) transpose ; DMA out ----
        yo = sb.tile([F, CT], FP, tag="dtT")
        for c0 in range(0, CT, NMM):
            po = ps.tile([F, NMM], FP, tag="m1a")
            nc.tensor.matmul(po[:], ct["w_out"][:], sz[:, c0:c0 + NMM],
                             start=True, stop=True)
            nc.scalar.copy(out=yo[:, c0:c0 + NMM], in_=po[:])
        yo4 = yo[:].rearrange("p (bl n t) -> p bl n t", n=n_c, t=T)
        TB = 4  # t's per out-staging tile
        for bl in range(bpb):
            for t0 in range(0, T, TB):
                pt = ps.tile([n_c, TB * F], FP, tag="tps")
                for k in range(TB):
                    nc.tensor.transpose(pt[:, k * F:(k + 1) * F],
                                        yo4[:, bl, :, t0 + k], ident[:F, :F])
                stg = sb2.tile([n_c, TB * F], FP, tag="ostg")
                nc.scalar.copy(out=stg[:], in_=pt[:])
                for k in range(TB):
                    nc.sync.dma_start(out_d[b0 + bl, t0 + k],
                                      stg[:, k * F:(k + 1) * F])
    es.close()


_CACHE = {}


def _get_program(key, consts, n_c, cmul_engine="vector"):
    if key not in _CACHE:
        _CACHE[key] = build_program(n_c, consts, cmul_engine)
    return _CACHE[key]


def kernel(**inputs):
    x = np.asarray(inputs["x"], np.float32)
    qk = np.asarray(inputs["qk"], np.float32)
    consts = _host_consts(inputs)
    n_c = N // NCORES

    nc = _get_program("main", consts, n_c)

    base = {
        "W1x": np.ascontiguousarray(consts["W1x"]),
        "W1z": np.ascontiguousarray(consts["W1z"]),
        "b1x": consts["b1x"].reshape(DI, 1).copy(),
        "b1z": consts["b1z"].reshape(DI, 1).copy(),
        "W2dt": np.ascontiguousarray(consts["W2dt"]),
        "W2bc": np.ascontiguousarray(consts["W2bc"]),
        "b_dt": consts["b_dt"].reshape(DI, 1).copy(),
        "conv_w": np.ascontiguousarray(consts["conv_w"]),
        "conv_b": consts["conv_b"].reshape(DI, 1).copy(),
        "D": consts["D"].reshape(DI, 1).copy(),
        "w_out": np.ascontiguousarray(consts["w_out"]),
    }
    in_maps = []
    for c in range(NCORES):
        sl = slice(c * n_c, (c + 1) * n_c)
        m = dict(base)
        m["x_sh"] = np.ascontiguousarray(x[:, :, sl, :])
        m["qk_sh"] = np.ascontiguousarray(qk[:, :, sl, :])
        in_maps.append(m)

    res = run_bass_kernel_spmd(nc, in_maps, core_ids=list(range(NCORES)))
    out = np.empty((B, T, N, F), np.float32)
    for c in range(NCORES):
        sl = slice(c * n_c, (c + 1) * n_c)
        out[:, :, sl, :] = res.results[c]["out_sh"].reshape(B, T, n_c, F)
    return out



# revision 13
# speedup vs baseline: 1.2899x; 1.2899x over previous
"""Trainium2 Bass kernel for CustomMamba (data-parallel over (b*n) scans).

Self-contained: builds + compiles a single-core SPMD Bass/Tile program,
shards inputs over 8 NeuronCores (along n), runs via run_bass_kernel_spmd,
and gathers the full output.

v2: debug/asserts off, bf16 scan-phase elementwise (DVE 2x mode), scan op
on GpSimd (frees DVE), merged d-halves (DH=128), activation table-set
batching (Silu + natural_log_exp only), no DRAM spills, batched out-DMA.
"""

import sys

sys.path.insert(0, "/opt/trn_rl_repo")

import os

os.environ.setdefault("JAX_PLATFORMS", "cpu")

from contextlib import ExitStack

import numpy as np

import concourse.bacc as bacc
import concourse.mybir as mybir
from concourse.bass_utils import run_bass_kernel_spmd
from concourse.masks import make_identity
from concourse.tile import TileContext

FP = mybir.dt.float32
BF = mybir.dt.bfloat16
AF = mybir.ActivationFunctionType
OP = mybir.AluOpType

# Problem constants (hardcoded per spec)
B, T, N, F = 8, 24, 512, 64
DI, DS, DR, DC = 128, 16, 4, 4
NCORES = 8


def _host_consts(inputs):
    """Fold the linear layers into per-stage weight matrices (fp32 numpy)."""
    w_mix = np.asarray(inputs["w_mix"], np.float32)      # [2F, F]
    b_mix = np.asarray(inputs["b_mix"], np.float32)      # [F]
    w_in = np.asarray(inputs["w_in"], np.float32)        # [F, 2*DI]
    conv_w = np.asarray(inputs["conv_w"], np.float32)    # [DI, DC]
    conv_b = np.asarray(inputs["conv_b"], np.float32)    # [DI]
    w_xproj = np.asarray(inputs["w_xproj"], np.float32)  # [DI, DR+2*DS]
    w_dt = np.asarray(inputs["w_dt"], np.float32)        # [DR, DI]
    b_dt = np.asarray(inputs["b_dt"], np.float32)        # [DI]
    A_log = np.asarray(inputs["A_log"], np.float32)      # [DI, DS]
    D = np.asarray(inputs["D"], np.float32)              # [DI]
    w_out = np.asarray(inputs["w_out"], np.float32)      # [DI, F]

    W1 = w_mix @ w_in                                    # [128, 2*DI]
    b1 = b_mix @ w_in                                    # [2*DI]
    W1x, W1z = W1[:, :DI].copy(), W1[:, DI:].copy()
    b1x, b1z = b1[:DI].copy(), b1[DI:].copy()

    W2dt = (w_xproj[:, :DR] @ w_dt).copy()               # [DI, DI]
    W2bc = w_xproj[:, DR:].copy()                        # [DI, 2*DS]

    A = -np.exp(A_log)                                   # [DI, DS]
    assert np.allclose(A, A[0:1, :], rtol=1e-6), "A varies across d"
    A_s = [float(A[0, s]) for s in range(DS)]

    return dict(
        W1x=W1x, W1z=W1z, b1x=b1x, b1z=b1z,
        W2dt=W2dt, W2bc=W2bc, b_dt=b_dt,
        conv_w=conv_w, conv_b=conv_b, D=D, w_out=w_out, A_s=A_s,
    )


def build_program(n_c, consts, scan_eng="gpsimd", x_eng="vector",
                  cmul_eng="vector", add_eng="vector", n_dve_scan=0):
    """Build + compile the per-core Bass program. n_c = n-shard width."""
    nc = bacc.Bacc(
        "TRN2",
        target_bir_lowering=False,
        debug=False,
        enable_asserts=False,
        num_devices=1,
    )

    bn = B * n_c
    ic = min(128, bn)
    nblk = bn // ic
    assert nblk * ic == bn
    bpb = ic // n_c                    # b's per block
    assert bpb * n_c == ic and bpb >= 1
    CT = ic * T

    x_d = nc.dram_tensor("x_sh", (B, T, n_c, F), FP, kind="ExternalInput").ap()
    qk_d = nc.dram_tensor("qk_sh", (B, T, n_c, F), FP, kind="ExternalInput").ap()
    cd = {}
    for nm, shp in [
        ("W1x", (2 * F, DI)), ("W1z", (2 * F, DI)),
        ("b1x", (DI, 1)), ("b1z", (DI, 1)),
        ("W2dt", (DI, DI)), ("W2bc", (DI, 2 * DS)), ("b_dt", (DI, 1)),
        ("conv_w", (DI, DC)), ("conv_b", (DI, 1)), ("D", (DI, 1)),
        ("w_out", (DI, F)),
    ]:
        cd[nm] = nc.dram_tensor(nm, shp, FP, kind="ExternalInput").ap()
    out_d = nc.dram_tensor("out_sh", (B, T, n_c, F), FP, kind="ExternalOutput").ap()

    with TileContext(nc) as tc:
        _body(nc, tc, x_d, qk_d, cd, out_d,
              n_c, ic, nblk, bpb, CT, consts,
              scan_eng, x_eng, cmul_eng, add_eng, n_dve_scan)
    nc.compile()
    return nc


def _body(nc, tc, x_d, qk_d, cd, out_d,
          n_c, ic, nblk, bpb, CT, consts,
          scan_eng, x_eng, cmul_eng, add_eng, n_dve_scan):
    P = ic
    NMM = 512 if CT % 512 == 0 else CT  # matmul N-chunk
    TG = 4                              # t's merged per transpose-psum tile
    use_b1 = not (np.allclose(consts["b1x"], 0) and np.allclose(consts["b1z"], 0))
    use_cb = not np.allclose(consts["conv_b"], 0)
    A_s = consts["A_s"]

    es = ExitStack()
    sb = es.enter_context(tc.tile_pool(name="sb", bufs=1))
    sb2 = es.enter_context(tc.tile_pool(name="sb2", bufs=2))
    ps = es.enter_context(tc.tile_pool(name="ps", bufs=2, space="PSUM"))

    # ---- constants ----
    ct = {}
    for nm in cd:
        t = sb.tile(list(cd[nm].shape), FP, tag=f"c_{nm}")
        nc.sync.dma_start(t[:], cd[nm])
        ct[nm] = t
    w_out_bf = sb.tile([DI, F], BF, tag="c_w_out_bf")
    nc.vector.tensor_copy(out=w_out_bf[:], in_=ct["w_out"][:])
    ident = sb.tile([128, 128], FP, tag="ident")
    make_identity(nc, ident[:])
    ident_bf = sb.tile([128, 128], BF, tag="ident_bf")
    nc.vector.tensor_copy(out=ident_bf[:], in_=ident[:])

    BSUB = min(bpb, 128 // T)          # b's per load/transpose chunk

    scan_p = nc.gpsimd if scan_eng == "gpsimd" else nc.vector
    xeng = nc.vector if x_eng == "vector" else nc.gpsimd
    ceng = nc.vector if cmul_eng == "vector" else nc.gpsimd
    aeng = nc.vector if add_eng == "vector" else nc.gpsimd

    for blk in range(nblk):
        b0 = blk * bpb

        # ---- load + transpose x/qk into xcatT [128=(fx|fqk), (i,t)] ----
        xcatT = sb.tile([128, CT], FP, tag="xcatT")
        NG = 8                         # n's per transpose-psum group
        NCH = 32                       # n's per raw DMA chunk
        for c0 in range(0, bpb, BSUB):
            bs = min(BSUB, bpb - c0)
            nrow = bs * T
            slot = 64 if nrow <= 64 else 128   # bank-aligned transpose slots
            for src_d, half, tagr in ((x_d, 0, "xraw"), (qk_d, 1, "qraw")):
                for nh in range(0, n_c, NCH):
                    ncw = min(NCH, n_c - nh)
                    raw = sb.tile([nrow, NCH * F], FP, tag=tagr)
                    nc.sync.dma_start(
                        raw[:, :ncw * F],
                        src_d[b0 + c0:b0 + c0 + bs, :, nh:nh + ncw].rearrange(
                            "b t n f -> (b t) (n f)"),
                    )
                    for g in range((ncw + NG - 1) // NG):
                        ng = min(NG, ncw - g * NG)
                        pt = ps.tile([F, NG * slot], FP, tag="tps")
                        for k in range(ng):
                            nc.tensor.transpose(
                                pt[:, k * slot:k * slot + nrow],
                                raw[:, (g * NG + k) * F:(g * NG + k + 1) * F],
                                ident[:nrow, :nrow],
                            )
                        # psum [F, (n ng, b bs, t T)] -> xcatT (b*n_c+n)*T+t
                        dst = xcatT[half * F:(half + 1) * F, :].rearrange(
                            "p (i t) -> p i t", t=T
                        ).rearrange("p (b n) t -> p n b t", b=bpb)[
                            :, nh + g * NG:nh + g * NG + ng, c0:c0 + bs]
                        src_ap = pt[:].rearrange(
                            "p (n r) -> p n r", r=slot)[:, :ng, :nrow].rearrange(
                            "p n (b t) -> p n b t", t=T)
                        nc.scalar.copy(out=dst, in_=src_ap)

        # ---- M1: xc = W1x.T @ xcatT ; z = W1z.T @ xcatT ----
        xc = sb.tile([DI, CT], FP, tag="xc")
        z = sb.tile([DI, CT], FP, tag="z")
        for c0 in range(0, CT, NMM):
            pxc = ps.tile([DI, NMM], FP, tag="m1a")
            pz = ps.tile([DI, NMM], FP, tag="m1b")
            nc.tensor.matmul(pxc[:], ct["W1x"][:], xcatT[:, c0:c0 + NMM],
                             start=True, stop=True)
            nc.tensor.matmul(pz[:], ct["W1z"][:], xcatT[:, c0:c0 + NMM],
                             start=True, stop=True)
            if use_b1:
                nc.scalar.activation(xc[:, c0:c0 + NMM], pxc[:], AF.Identity,
                                     bias=ct["b1x"][:, 0:1])
                nc.scalar.activation(z[:, c0:c0 + NMM], pz[:], AF.Identity,
                                     bias=ct["b1z"][:, 0:1])
            else:
                nc.scalar.copy(out=xc[:, c0:c0 + NMM], in_=pxc[:])
                nc.scalar.copy(out=z[:, c0:c0 + NMM], in_=pz[:])

        # ---- causal depthwise conv (+bias) ----
        acc = sb.tile([DI, CT], FP, tag="xcatT")   # xcatT dead after M1
        nc.scalar.mul(acc[:], xc[:], ct["conv_w"][:, DC - 1:DC])
        xc3 = xc[:].rearrange("p (i t) -> p i t", t=T)
        ac3 = acc[:].rearrange("p (i t) -> p i t", t=T)
        for k in range(DC - 1):
            d = DC - 1 - k
            nc.vector.scalar_tensor_tensor(
                out=ac3[:, :, d:], in0=xc3[:, :, :T - d],
                scalar=ct["conv_w"][:, k:k + 1],
                in1=ac3[:, :, d:], op0=OP.mult, op1=OP.add,
            )
        if use_cb:
            nc.scalar.activation(acc[:], acc[:], AF.Identity,
                                 bias=ct["conv_b"][:, 0:1])
        # silu via the HW Silu table (batch both Silu ops: one table set)
        xc2 = sb.tile([DI, CT], FP, tag="xc2")
        nc.scalar.activation(xc2[:], acc[:], AF.Silu)
        sz = sb2.tile([DI, CT], BF, tag="sz")
        nc.scalar.activation(sz[:], z[:], AF.Silu)
        # xcD = xc2 * D (bf16, for gating later)
        xcD = sb2.tile([DI, CT], BF, tag="xcD")
        nc.scalar.activation(xcD[:], xc2[:], AF.Identity, scale=ct["D"][:, 0:1])

        # ---- M2: dt = softplus(W2dt.T @ xc2 + b_dt); bc = W2bc.T @ xc2 ----
        dt = sb.tile([DI, CT], FP, tag="z")      # reuse z slot
        bc = sb.tile([2 * DS, CT], BF, tag="m2tmp")
        for c0 in range(0, CT, NMM):
            pdt = ps.tile([DI, NMM], FP, tag="m1a")
            pbc = ps.tile([2 * DS, NMM], FP, tag="m1b")
            nc.tensor.matmul(pdt[:], ct["W2dt"][:], xc2[:, c0:c0 + NMM],
                             start=True, stop=True)
            nc.tensor.matmul(pbc[:], ct["W2bc"][:], xc2[:, c0:c0 + NMM],
                             start=True, stop=True)
            # softplus(x + b_dt) = ln(1 + exp(x + b_dt)) (exp/ln share a set)
            spe = sb2.tile([DI, NMM], FP, tag="spe")
            nc.scalar.activation(spe[:], pdt[:], AF.Exp,
                                 bias=ct["b_dt"][:, 0:1])
            nc.scalar.activation(dt[:, c0:c0 + NMM], spe[:], AF.Ln, bias=1.0)
            nc.scalar.copy(out=bc[:, c0:c0 + NMM], in_=pbc[:])

        # du = dt * xc2 (bf16 out; fp32 reads)
        duf = sb.tile([DI, CT], BF, tag="duf")
        nc.vector.tensor_tensor(duf[:], dt[:], xc2[:], OP.mult)

        # ---- transposes into scan layout [i, (d, t)] ----
        dtT = sb.tile([P, DI * T], FP, tag="dtT")
        duT = sb.tile([P, DI * T], BF, tag="duT")
        bcT = sb.tile([P, 2 * DS * T], BF, tag="bcT")
        for (srct, dstt, rows, idn, pdt) in (
                (dt, dtT, DI, ident, FP), (duf, duT, DI, ident_bf, BF),
                (bc, bcT, 2 * DS, ident_bf, BF)):
            s3 = srct[:].rearrange("p (i t) -> p i t", t=T)
            for t0 in range(0, T, TG):
                pt = ps.tile([P, TG * rows], pdt, tag="tps")
                for k in range(TG):
                    nc.tensor.transpose(
                        pt[:, k * rows:(k + 1) * rows],
                        s3[:rows, :, t0 + k],
                        idn[:rows, :rows],
                    )
                dst = dstt[:].rearrange("p (d t) -> p d t", t=T)[:, :, t0:t0 + TG]
                nc.scalar.copy(
                    out=dst, in_=pt[:].rearrange("p (t d) -> p d t", t=TG))

        # ---- scan phase: one pass per state s over [i, (d=128, t=24)] ----
        duT3 = duT[:].rearrange("p (d t) -> p d t", t=T)
        bcT3 = bcT[:].rearrange("p (c t) -> p c t", t=T)
        ya = None
        for s in range(DS):
            dA = sb2.tile([P, DI * T], FP, tag="dA")
            nc.scalar.activation(dA[:], dtT[:], AF.Exp, scale=A_s[s])
            dA3 = dA[:].rearrange("p (d t) -> p d t", t=T)
            nc.gpsimd.memset(dA3[:, :, 0:1], 0.0)
            Xs = sb2.tile([P, DI * T], BF, tag="Xs")
            xeng.tensor_tensor(
                Xs[:].rearrange("p (d t) -> p d t", t=T),
                duT3[:, :, :],
                bcT3[:, s:s + 1, :].to_broadcast((P, DI, T)),
                OP.mult,
            )
            hs = sb2.tile([P, DI * T], BF, tag="hs")
            sp = nc.vector if s < n_dve_scan else scan_p
            sp.tensor_tensor_scan(hs[:], dA[:], Xs[:], 0.0, OP.mult, OP.add)
            tmp = sb2.tile([P, DI * T], BF, tag="Xs")
            ceng.tensor_tensor(
                tmp[:].rearrange("p (d t) -> p d t", t=T),
                hs[:].rearrange("p (d t) -> p d t", t=T),
                bcT3[:, DS + s:DS + s + 1, :].to_broadcast((P, DI, T)),
                OP.mult,
            )
            yb = sb2.tile([P, DI * T], BF, tag="yp")
            if ya is None:
                nc.vector.tensor_copy(out=yb[:], in_=tmp[:])
            else:
                aeng.tensor_tensor(yb[:], ya[:], tmp[:], OP.add)
            ya = yb

        # ---- transpose y back: [i,(d,t)] -> y_d [d,(i,t)] (bf16) ----
        y_d = sb.tile([DI, CT], BF, tag="y_d")
        ya3 = ya[:].rearrange("p (d t) -> p d t", t=T)
        for t0 in range(0, T, TG):
            pt = ps.tile([DI, TG * P], BF, tag="tps")
            for k in range(TG):
                nc.tensor.transpose(pt[:, k * P:(k + 1) * P],
                                    ya3[:, :, t0 + k], ident_bf[:P, :P])
            dst = y_d[:, :].rearrange(
                "p (i t) -> p i t", t=T)[:, :, t0:t0 + TG]
            nc.scalar.copy(out=dst,
                           in_=pt[:].rearrange("p (t i) -> p i t", t=TG))

        # ---- gate: y2 = (y_d + xcD) * sz  (bf16) ----
        t2 = sb.tile([DI, CT], BF, tag="xc")     # xc dead after conv
        nc.vector.tensor_tensor(t2[:], y_d[:], xcD[:], OP.add)
        y2 = sb.tile([DI, CT], BF, tag="duf")    # duf slot free now
        nc.vector.tensor_tensor(y2[:], t2[:], sz[:], OP.mult)

        # ---- M3: out = w_out.T @ y2 (bf16) ; transpose ; DMA out ----
        yo = sb.tile([F, CT], BF, tag="m2tmp")   # bc slot free now
        for c0 in range(0, CT, NMM):
            po = ps.tile([F, NMM], FP, tag="m1a")
            nc.tensor.matmul(po[:], w_out_bf[:], y2[:, c0:c0 + NMM],
                             start=True, stop=True)
            nc.scalar.copy(out=yo[:, c0:c0 + NMM], in_=po[:])
        yo4 = yo[:].rearrange("p (bl n t) -> p bl n t", n=n_c, t=T)
        TB = 4  # t's per out-transpose psum tile
        for bl in range(bpb):
            stg = sb.tile([n_c, T * F], FP, tag="ostg")
            for t0 in range(0, T, TB):
                pt = ps.tile([n_c, TB * F], BF, tag="tps")
                for k in range(TB):
                    nc.tensor.transpose(pt[:, k * F:(k + 1) * F],
                                        yo4[:, bl, :, t0 + k],
                                        ident_bf[:F, :F])
                nc.scalar.copy(out=stg[:, t0 * F:(t0 + TB) * F], in_=pt[:])
            nc.sync.dma_start(
                out_d[b0 + bl].rearrange("t n f -> n t f"),
                stg[:].rearrange("n (t f) -> n t f", f=F))
    es.close()


_CACHE = {}


def _get_program(key, consts, n_c, **kw):
    if key not in _CACHE:
        _CACHE[key] = build_program(n_c, consts, **kw)
    return _CACHE[key]


def kernel(**inputs):
    x = np.asarray(inputs["x"], np.float32)
    qk = np.asarray(inputs["qk"], np.float32)
    consts = _host_consts(inputs)
    n_c = N // NCORES

    nc = _get_program("main", consts, n_c)

    base = {
        "W1x": np.ascontiguousarray(consts["W1x"]),
        "W1z": np.ascontiguousarray(consts["W1z"]),
        "b1x": consts["b1x"].reshape(DI, 1).copy(),
        "b1z": consts["b1z"].reshape(DI, 1).copy(),
        "W2dt": np.ascontiguousarray(consts["W2dt"]),
        "W2bc": np.ascontiguousarray(consts["W2bc"]),
        "b_dt": consts["b_dt"].reshape(DI, 1).copy(),
        "conv_w": np.ascontiguousarray(consts["conv_w"]),
        "conv_b": consts["conv_b"].reshape(DI, 1).copy(),
        "D": consts["D"].reshape(DI, 1).copy(),
        "w_out": np.ascontiguousarray(consts["w_out"]),
    }
    in_maps = []
    for c in range(NCORES):
        sl = slice(c * n_c, (c + 1) * n_c)
        m = dict(base)
        m["x_sh"] = np.ascontiguousarray(x[:, :, sl, :])
        m["qk_sh"] = np.ascontiguousarray(qk[:, :, sl, :])
        in_maps.append(m)

    res = run_bass_kernel_spmd(nc, in_maps, core_ids=list(range(NCORES)))
    out = np.empty((B, T, N, F), np.float32)
    for c in range(NCORES):
        sl = slice(c * n_c, (c + 1) * n_c)
        out[:, :, sl, :] = res.results[c]["out_sh"].reshape(B, T, n_c, F)
    return out
